# revision 1
# baseline (speedup 1.0000x reference)
"""ContraCLM token-level contrastive loss on 8 Trainium2 NeuronCores.

Data-parallel over the batch: core b handles sample b (B=8). Per core,
with S=1536, D=1024, T=0.05:

  f_v = l2norm(h_v) with masked token rows zeroed (mask folded into the
  rsqrt scale);  F = [f1; f2]  (2S x D, bf16, stored transposed as [D, 2S])

  sim = F F^T computed as 24 x 6 grid of [128, 512] PSUM strips (K=1024).
  exp(sim/T) row sums come free from the ScalarE activation free-dim
  accumulator. Diagonal-block strips (self-sim and positive-counterpart
  entries, which land on 128-block diagonals because 2S is a multiple of
  128 and partner offset is S) get the diagonal zeroed via affine_select
  before a DVE row-sum instead.

  Masked columns were zeroed in F, so each masked column contributes
  exp(0)=1 to a row sum: subtract K0 = 2S - 2n afterwards.
  pos_sim is computed exactly in fp32 as a row-wise dot product.
  per_tok = log(Ng + exp(pos_sim/T)) - pos_sim/T; masked mean over 2n
  tokens; AllReduce-mean across the 8 cores.
"""

import sys

for _p in ("/opt/trn_rl_repo", "/opt/pypackages"):
    if _p not in sys.path:
        sys.path.append(_p)

from contextlib import ExitStack

import numpy as np

import bass_rust

import concourse.bass as bass
import concourse.tile as tile
from concourse import mybir
from concourse.bass_utils import run_bass_kernel_spmd
from concourse.masks import make_identity
from concourse.vector_clock import ScopedClock

# The walrus build in this container encodes at most 2 sync waits per
# instruction (bass_rust's inst_waits_full agrees), but Tile's semaphore
# assignment can attach more. Hoist excess waits onto unfusable same-engine
# NoOps immediately before the instruction — the engine executes its queue
# in order, so semantics are preserved.
_MAX_WAITS = 1


def _split_excess_waits(nc, ordered):
    for bb_name, insts in ordered.items():
        out = []
        changed = False
        for inst in insts:
            si = getattr(inst, "sync_info", None)
            waits = list(si.on_wait) if si is not None else []
            if len(waits) > _MAX_WAITS:
                changed = True
                extra, keep = waits[:-_MAX_WAITS], waits[-_MAX_WAITS:]
                for i in range(0, len(extra), _MAX_WAITS):
                    out.append(mybir.InstNoOp(
                        name=nc.get_next_instruction_name(),
                        sync_info=mybir.SyncInfo(
                            on_wait=extra[i:i + _MAX_WAITS], on_update=[]),
                        bass_nofuse=True,
                        engine=inst.engine,
                    ))
                si.on_wait = keep
            out.append(inst)
        if changed:
            insts[:] = out


_orig_lower_ordered_insts = tile.TileContext._lower_ordered_insts


def _patched_lower_ordered_insts(self, ordered):
    _split_excess_waits(self.nc, ordered)
    return _orig_lower_ordered_insts(self, ordered)


tile.TileContext._lower_ordered_insts = _patched_lower_ordered_insts


def _split_waits_drain_and_barrier(self, tick_clock, wait_clock):
    nc = self.nc
    probe = nc.sync.nop(nofuse=True)
    wait_clock.add_sem_waits(
        probe.ins, ScopedClock({None: tick_clock.global_clock}))
    si = probe.ins.sync_info
    waits = list(si.on_wait) if si is not None else []
    if len(waits) > _MAX_WAITS:
        si.on_wait = waits[:_MAX_WAITS]
        for i in range(_MAX_WAITS, len(waits), _MAX_WAITS):
            nxt = nc.sync.nop(nofuse=True)
            nxt.ins.sync_info = bass_rust.SyncInfo(
                on_wait=waits[i:i + _MAX_WAITS], on_update=[])
    nc.sync.drain()
    nc.all_engine_barrier()
    assert self.sems is not None
    popped = nc._tile_sem_poison_stack.pop()
    assert popped is self._sem_poison
    nc.clear_and_free_semaphores(list(self.sems.allocated().values()))
    nc.all_engine_barrier()


tile.TileContext._drain_and_barrier = _split_waits_drain_and_barrier

S, D, NCORES = 1536, 1024, 8
ST = S // 128            # 12 s-tiles per view
NB = 2 * ST              # 24 block rows of F
NCS = 2 * S // 512       # 6 column strips
KT = D // 128            # 8 contraction tiles
TEMP_INV = 20.0          # 1 / 0.05
FP8_SCALE = 8.0          # f entries ~N(0, 1/32); x8 keeps them in e4m3's
                         # normal range (|f|*8 <~ 2, well under 240)
F32 = mybir.dt.float32
BF16 = mybir.dt.bfloat16
FP8 = mybir.dt.float8e4
AF = mybir.ActivationFunctionType
ALU = mybir.AluOpType


def _build(num_devices: int = NCORES, debug_dump: bool = False) -> bass.Bass:
    nc = bass.Bass(num_devices=num_devices)
    h1 = nc.dram_tensor("h1", [S, D], F32, kind="ExternalInput")
    h2 = nc.dram_tensor("h2", [S, D], F32, kind="ExternalInput")
    # mask, pre-laid-out host-side as [128, ST] so token t = 128*col + row
    maskT = nc.dram_tensor("maskT", [128, ST], F32, kind="ExternalInput")
    out = nc.dram_tensor("loss", [1, 1], F32, kind="ExternalOutput")
    if debug_dump:
        ng_dump = nc.dram_tensor("ng_dump", [128, NB], F32,
                                 kind="ExternalOutput")
        cacc_dump = nc.dram_tensor("cacc_dump", [128, ST], F32,
                                   kind="ExternalOutput")

    exp_scale = TEMP_INV / (FP8_SCALE * FP8_SCALE)

    with tile.TileContext(nc) as tc, ExitStack() as ctx:
        const_pool = ctx.enter_context(tc.tile_pool(name="const", bufs=1))
        big = ctx.enter_context(tc.tile_pool(name="big", bufs=1))
        stat = ctx.enter_context(tc.tile_pool(name="stat", bufs=1))

        ident = const_pool.tile([128, 128], BF16)
        make_identity(nc, ident[:])
        ones_col = const_pool.tile([128, 1], F32)
        nc.gpsimd.memset(ones_col[:], 1.0)
        ones_sq = const_pool.tile([128, 128], F32)
        nc.gpsimd.memset(ones_sq[:], 1.0)
        ones_bf = const_pool.tile([128, 1], BF16)
        nc.gpsimd.memset(ones_bf[:], 1.0)
        msk = const_pool.tile([128, ST], F32)
        nc.sync.dma_start(msk[:], maskT[:])

        fT1 = big.tile([128, KT, S], FP8)        # f1^T * 8, fp8e4
        fT2 = big.tile([128, KT, S], FP8)        # f2^T * 8
        h1keep = big.tile([128, ST, D], F32)     # raw h1, kept for pos dot
        s12 = stat.tile([128, ST], F32)          # raw <h1_i, h2_i>
        poss = stat.tile([128, ST], F32)         # pos_sim (masked rows -> 0)
        sc1buf = stat.tile([128, ST], F32)       # masked rsqrt scales view1
        acc = stat.tile([128, NB, NCS], F32)     # per-strip row sums
        cacc = stat.tile([128, ST], F32)         # B column sums (view-2 Ng)
        msk24 = stat.tile([128, NB], F32)
        pose24 = stat.tile([128, NB], F32)       # exp(pos_sim/T), doubled
        poss20m = stat.tile([128, NB], F32)      # mask * pos_sim/T, doubled
        negK0 = stat.tile([128, 1], F32)
        recn = stat.tile([1, 1], F32)

        # acc[view2 rows, A-col-strips] is never written; zero everything
        nc.gpsimd.memset(acc[:], 0.0)

        # ---- phase 0: mask-only precomputes ----
        with tc.tile_pool(name="ep0", bufs=1) as ep0, \
             tc.tile_pool(name="ep0_ps", bufs=1, space="PSUM") as ep0p:
            msum = ep0.tile([128, 1], F32)
            nc.vector.tensor_reduce(msum[:], msk[:],
                                    axis=mybir.AxisListType.X, op=ALU.add)
            nps = ep0p.tile([128, 1], F32)
            nc.tensor.matmul(nps[:], ones_sq[:], msum[:], start=True,
                             stop=True)
            # -K0 = 2n - 2S
            nc.scalar.activation(negK0[:], nps[:], AF.Copy, scale=2.0,
                                 bias=float(-2 * S))
            n2c = ep0.tile([1, 1], F32)
            nc.scalar.activation(n2c[:], nps[0:1, :], AF.Copy,
                                 scale=float(2 * num_devices))
            nc.vector.reciprocal(recn[:], n2c[:])   # 1/(2 n ncores)
            nc.vector.tensor_copy(msk24[:, 0:ST], msk[:])
            nc.vector.tensor_copy(msk24[:, ST:NB], msk[:])

        def load_view(t, dst, src_dram, keep):
            if keep is not None:
                ht = keep
                nc.sync.dma_start(ht[:], src_dram[t * 128:(t + 1) * 128, :])
            else:
                nc.sync.dma_start(dst[:], src_dram[t * 128:(t + 1) * 128, :])
                ht = dst
            return ht

        def norms_of(scp, scr, ht, t, tag):
            sq = scr.tile([128, D], BF16, tag="sq", name=f"sq_{tag}_{t}")
            ss = scp.tile([128, 1], F32, tag="ss", name=f"ss_{tag}_{t}")
            nc.scalar.activation(sq[:], ht[:], AF.Square, accum_out=ss[:])
            nrm = scp.tile([128, 1], F32, tag="nrm", name=f"nrm_{tag}_{t}")
            nc.scalar.sqrt(nrm[:], ss[:])
            ri = scp.tile([128, 1], F32, tag="ri", name=f"ri_{tag}_{t}")
            nc.vector.reciprocal(ri[:], nrm[:])
            sc = scp.tile([128, 1], F32, tag="msc", name=f"sc_{tag}_{t}")
            nc.vector.tensor_mul(sc[:], ri[:], msk[:, t:t + 1])
            return sc

        def normalize_transpose(scr, tps, ht, sc, fT, t, tag):
            fn = scr.tile([128, D], BF16, tag="fn", name=f"fn_{tag}_{t}")
            nc.vector.tensor_scalar_mul(fn[:], ht[:], sc[:])
            c0 = t * 128
            for kg in range(2):
                pt = tps.tile([128, 512], BF16, tag="pt", name=f"pt_{tag}_{t}_{kg}")
                for j in range(4):
                    k = kg * 4 + j
                    nc.tensor.transpose(pt[:, j * 128:(j + 1) * 128],
                                        fn[:, k * 128:(k + 1) * 128],
                                        ident[:])
                nc.vector.tensor_scalar_mul(
                    fT[:, kg * 4:(kg + 1) * 4, c0:c0 + 128],
                    pt[:].rearrange("p (j c) -> p j c", j=4),
                    FP8_SCALE)

        def strip(mmp, esp, cs, r):
            """One [128,512] sim strip: matmuls, exp, row-sum into acc."""
            lhsT = fT1 if r < ST else fT2
            rT = r % ST
            rhsT = fT1 if cs < NCS // 2 else fT2
            csT = cs % (NCS // 2)
            ps = mmp.tile([128, 512], F32, tag="ps", name=f"ps_{cs}_{r}")
            for g in range(KT // 2):
                nc.tensor.matmul(
                    ps[:],
                    lhsT[:, 2 * g:2 * g + 2, rT * 128:(rT + 1) * 128],
                    rhsT[:, 2 * g:2 * g + 2, csT * 512:(csT + 1) * 512],
                    perf_mode=mybir.MatmulPerfMode.DoubleRow,
                    start=(g == 0), stop=(g == KT // 2 - 1))
            es = esp.tile([128, 512], BF16, tag="es", name=f"es_{cs}_{r}")
            bad = [bc for bc in (r % ST, r % ST + ST)
                   if cs * 4 <= bc < cs * 4 + 4]
            if bad:
                jb = bad[0] - cs * 4
                nc.scalar.activation(es[:], ps[:], AF.Exp, scale=exp_scale)
                blk = es[:, jb * 128:(jb + 1) * 128]
                nc.gpsimd.affine_select(
                    out=blk, in_=blk, compare_op=ALU.not_equal,
                    fill=0.0, base=0, pattern=[[-1, 128]],
                    channel_multiplier=1)
                nc.vector.tensor_reduce(acc[:, r, cs:cs + 1], es[:],
                                        axis=mybir.AxisListType.X,
                                        op=ALU.add)
            else:
                nc.scalar.activation(es[:], ps[:], AF.Exp, scale=exp_scale,
                                     accum_out=acc[:, r, cs:cs + 1])
            return es

        with tc.tile_pool(name="mm_ps", bufs=3, space="PSUM") as mmp, \
             tc.tile_pool(name="es", bufs=3) as esp, \
             tc.tile_pool(name="scr", bufs=2) as scr, \
             tc.tile_pool(name="sc", bufs=4) as scp:

            # ---- phase A: view-1 load/normalize/transpose ----
            with tc.tile_pool(name="tpA_ps", bufs=3, space="PSUM") as tps:
                for t in range(ST):
                    ht = load_view(t, None, h1, h1keep[:, t, :])
                    sc1 = norms_of(scp, scr, ht, t, "a")
                    nc.vector.tensor_copy(sc1buf[:, t:t + 1], sc1[:])
                    normalize_transpose(scr, tps, ht, sc1, fT1, t, "a")

            # ---- phase A': A-quadrant strips (only need view 1) ----
            for cs in range(NCS // 2):
                for r in range(ST):
                    strip(mmp, esp, cs, r)

            # ---- phase B: view-2 load/normalize/transpose + pos dot ----
            with tc.tile_pool(name="tpB_ps", bufs=3, space="PSUM") as tps, \
                 tc.tile_pool(name="ldB", bufs=3) as ldB:
                for t in range(ST):
                    tb = ldB.tile([128, D], F32, tag="h2", name=f"h2_{t}")
                    load_view(t, tb, h2, None)
                    sc2 = norms_of(scp, scr, tb, t, "b")
                    prod = scr.tile([128, D], F32, tag="prod",
                                    name=f"prod_{t}")
                    nc.vector.tensor_mul(prod[:], h1keep[:, t, :], tb[:])
                    nc.vector.tensor_reduce(s12[:, t:t + 1], prod[:],
                                            axis=mybir.AxisListType.X,
                                            op=ALU.add)
                    ptmp = scp.tile([128, 1], F32, tag="ptmp",
                                    name=f"ptmp_{t}")
                    nc.vector.tensor_mul(ptmp[:], s12[:, t:t + 1],
                                         sc1buf[:, t:t + 1])
                    nc.vector.tensor_mul(poss[:, t:t + 1], ptmp[:], sc2[:])
                    normalize_transpose(scr, tps, tb, sc2, fT2, t, "b")

            # pos-dependent epilogue precomputes (overlap with B/C strips)
            nc.scalar.activation(pose24[:, 0:ST], poss[:], AF.Exp,
                                 scale=TEMP_INV)
            nc.scalar.activation(pose24[:, ST:NB], poss[:], AF.Exp,
                                 scale=TEMP_INV)
            p20 = stat.tile([128, ST], F32)
            nc.scalar.mul(p20[:], poss[:], TEMP_INV)
            nc.vector.tensor_mul(poss20m[:, 0:ST], p20[:], msk[:])
            nc.vector.tensor_copy(poss20m[:, ST:NB], poss20m[:, 0:ST])

            # ---- phase B': B and C strips + B column sums ----
            with tc.tile_pool(name="cb_ps", bufs=1, space="PSUM") as cbp:
                for cs in range(NCS // 2, NCS):
                    pcb = []
                    for jb in range(4):
                        pcb_jb = cbp.tile([128, 1], F32, tag=f"cb{jb}",
                                          name=f"pcb_{cs}_{jb}")
                        pcb.append(pcb_jb)
                    for r in range(NB):
                        es = strip(mmp, esp, cs, r)
                        if r < ST:
                            for jb in range(4):
                                nc.tensor.matmul(
                                    pcb[jb][:],
                                    es[:, jb * 128:(jb + 1) * 128],
                                    ones_bf[:],
                                    start=(r == 0), stop=(r == ST - 1),
                                    skip_group_check=True)
                    c0 = (cs - NCS // 2) * 4
                    for jb in range(4):
                        nc.vector.tensor_copy(cacc[:, c0 + jb:c0 + jb + 1],
                                              pcb[jb][:])

        # ---- phase C: final reduction chain ----
        with tc.tile_pool(name="ep", bufs=1) as ep, \
             tc.tile_pool(name="ep_ps", bufs=1, space="PSUM") as epp:
            ng = ep.tile([128, NB], F32)
            nc.vector.tensor_reduce(ng[:], acc[:], axis=mybir.AxisListType.X,
                                    op=ALU.add)
            nc.vector.tensor_add(ng[:, ST:NB], ng[:, ST:NB], cacc[:])
            if debug_dump:
                nc.sync.dma_start(ng_dump[:], ng[:])
                nc.sync.dma_start(cacc_dump[:], cacc[:])
            denom = ep.tile([128, NB], F32)
            nc.vector.tensor_scalar_add(denom[:], ng[:], negK0[:])
            nc.vector.tensor_add(denom[:], denom[:], pose24[:])
            lg = ep.tile([128, NB], F32)
            nc.scalar.activation(lg[:], denom[:], AF.Ln)
            ptok = ep.tile([128, NB], F32)
            nc.vector.tensor_mul(ptok[:], lg[:], msk24[:])
            nc.vector.tensor_sub(ptok[:], ptok[:], poss20m[:])
            tsum = ep.tile([128, 1], F32)
            nc.vector.tensor_reduce(tsum[:], ptok[:],
                                    axis=mybir.AxisListType.X, op=ALU.add)
            lps = epp.tile([1, 1], F32)
            nc.tensor.matmul(lps[:], ones_col[:], tsum[:], start=True,
                             stop=True)
            lsb = ep.tile([1, 1], F32)
            nc.vector.tensor_mul(lsb[:], lps[:], recn[:])

            with tc.tile_pool(name="dram", bufs=1, space="DRAM") as dram:
                if num_devices > 1:
                    lin = dram.tile([1, 1], F32)
                    lout = dram.tile([1, 1], F32)
                    nc.sync.dma_start(lin[:], lsb[:])
                    nc.gpsimd.collective_compute(
                        "AllReduce", ALU.add,
                        replica_groups=[list(range(num_devices))],
                        ins=[lin.opt()], outs=[lout.opt()])
                    nc.sync.dma_start(out[:], lout[:])
                else:
                    nc.sync.dma_start(out[:], lsb[:])

    return nc


_NC = None


def _mask_layout(mask_row: np.ndarray) -> np.ndarray:
    # token t = 128 * col + row  ->  [128, ST]
    return np.ascontiguousarray(
        mask_row.astype(np.float32).reshape(ST, 128).T)


def kernel(last_hidden_states_1, last_hidden_states_2, token_mask_batch):
    global _NC
    h1 = np.ascontiguousarray(np.asarray(last_hidden_states_1,
                                         dtype=np.float32))
    h2 = np.ascontiguousarray(np.asarray(last_hidden_states_2,
                                         dtype=np.float32))
    mask = np.asarray(token_mask_batch)
    assert h1.shape == (NCORES, S, D), h1.shape

    if _NC is None:
        _NC = _build(NCORES)

    in_maps = [
        {"h1": h1[b], "h2": h2[b], "maskT": _mask_layout(mask[b])}
        for b in range(NCORES)
    ]
    res = run_bass_kernel_spmd(_NC, in_maps, list(range(NCORES)))
    loss = np.asarray(res.results[0]["loss"], dtype=np.float32).reshape(())
    return loss



# revision 6
# speedup vs baseline: 2.2036x; 2.2036x over previous
"""ContraCLM token-level contrastive loss on 8 Trainium2 NeuronCores.

Data-parallel over the batch: core b handles sample b (B=8). The token
mask is known on the host inside kernel(), so unmasked tokens are
COMPACTED host-side to a fixed padded length NCOMP (= 896 >= n + 6.5
sigma for n ~ Binom(1536, 1/2)); pad slots reuse token 0's row and get
mask 0. The device then works on a dense [2*NCOMP, 2*NCOMP] problem --
2.9x fewer matmul/exp elements than the full 2S grid.

Per core, with N_c = NCOMP, D = 1024, T = 0.05:

  f_v = l2norm(h_v) masked; rsqrt computed as exp(-0.5*ln(ss)) so the
  ScalarE only ever needs the natural_log_exp activation-table set.
  F = [f1; f2], stored transposed as fp8 [D, 2*N_c] (x8 scale).

  sim strips: per row-block r, one PSUM group [128, N_c] (<=2 banks)
  accumulates 4 DoubleRow fp8 matmuls per 512-col strip (K=1024).
  Self-similarity diagonals are killed by ONE extra tiny matmul that
  accumulates -30000*I into the diagonal 128-block, so exp() of the
  whole group rowsum-accumulates on the ScalarE free-dim accumulator
  with no fixups. The positive-pair column (block diagonal of the B
  quadrant) stays IN the row sum (denominator = Ng + pos), and pos
  itself is extracted from the exp'd block via affine_select(diag).

  C-quadrant row sums (view-2 rows vs view-1 cols) are B-quadrant
  column sums by symmetry: ones-weight matmuls accumulate [1, 512]
  PSUM rows which are transposed back to token layout on the PE.

  Masked/pad columns contribute exp(0)=1 to every row sum: corrected
  with negK0 = 2n - 2*N_c (host-computed, folded into the Ln bias).
  per-core output = sum_tok mask*(ln(Ng+pos) - ln(pos)); the host
  divides by 2n and averages across the 8 cores (no device collective).
"""

import sys

for _p in ("/opt/trn_rl_repo", "/opt/pypackages"):
    if _p not in sys.path:
        sys.path.append(_p)

from contextlib import ExitStack

import numpy as np

import bass_rust

import concourse.bass as bass
import concourse.tile as tile
from concourse import mybir
from concourse.bass_utils import run_bass_kernel_spmd
from concourse.masks import make_identity
from concourse.vector_clock import ScopedClock

# The walrus build in this container encodes at most 2 sync waits per
# instruction (bass_rust's inst_waits_full agrees), but Tile's semaphore
# assignment can attach more. Hoist excess waits onto unfusable same-engine
# NoOps immediately before the instruction -- the engine executes its queue
# in order, so semantics are preserved.
_MAX_WAITS = 1


def _split_excess_waits(nc, ordered):
    for bb_name, insts in ordered.items():
        out = []
        changed = False
        for inst in insts:
            si = getattr(inst, "sync_info", None)
            waits = list(si.on_wait) if si is not None else []
            if len(waits) > _MAX_WAITS:
                changed = True
                extra, keep = waits[:-_MAX_WAITS], waits[-_MAX_WAITS:]
                for i in range(0, len(extra), _MAX_WAITS):
                    out.append(mybir.InstNoOp(
                        name=nc.get_next_instruction_name(),
                        sync_info=mybir.SyncInfo(
                            on_wait=extra[i:i + _MAX_WAITS], on_update=[]),
                        bass_nofuse=True,
                        engine=inst.engine,
                    ))
                si.on_wait = keep
            out.append(inst)
        if changed:
            insts[:] = out


_orig_lower_ordered_insts = tile.TileContext._lower_ordered_insts


def _patched_lower_ordered_insts(self, ordered):
    _split_excess_waits(self.nc, ordered)
    return _orig_lower_ordered_insts(self, ordered)


tile.TileContext._lower_ordered_insts = _patched_lower_ordered_insts


def _split_waits_drain_and_barrier(self, tick_clock, wait_clock):
    nc = self.nc
    probe = nc.sync.nop(nofuse=True)
    wait_clock.add_sem_waits(
        probe.ins, ScopedClock({None: tick_clock.global_clock}))
    si = probe.ins.sync_info
    waits = list(si.on_wait) if si is not None else []
    if len(waits) > _MAX_WAITS:
        si.on_wait = waits[:_MAX_WAITS]
        for i in range(_MAX_WAITS, len(waits), _MAX_WAITS):
            nxt = nc.sync.nop(nofuse=True)
            nxt.ins.sync_info = bass_rust.SyncInfo(
                on_wait=waits[i:i + _MAX_WAITS], on_update=[])
    nc.sync.drain()
    nc.all_engine_barrier()
    assert self.sems is not None
    popped = nc._tile_sem_poison_stack.pop()
    assert popped is self._sem_poison
    nc.clear_and_free_semaphores(list(self.sems.allocated().values()))
    nc.all_engine_barrier()


tile.TileContext._drain_and_barrier = _split_waits_drain_and_barrier

S, D, NCORES = 1536, 1024, 8
NCOMP = 896              # padded compacted token count (multiple of 128)
KT = D // 128            # 8 contraction k-tiles
TEMP_INV = 20.0          # 1 / 0.05
FP8_SCALE = 8.0          # f entries ~N(0, 1/32); x8 keeps them in e4m3 range
NEGBIG = -30000.0        # diag killer: exp(scale*(64*sim + NEGBIG)) == 0
F32 = mybir.dt.float32
BF16 = mybir.dt.bfloat16
FP8 = mybir.dt.float8e4
AF = mybir.ActivationFunctionType
ALU = mybir.AluOpType


def _col_strips(sc):
    """Split sc (= NCOMP, multiple of 128) into <=512-wide col strips."""
    strips = []
    c = 0
    while c < sc:
        w = min(512, sc - c)
        strips.append((c, w))
        c += w
    return strips


def _build(ncomp: int) -> bass.Bass:
    st = ncomp // 128          # token tiles per view
    nb = 2 * st                # row blocks of F
    strips = _col_strips(ncomp)
    exp_scale = TEMP_INV / (FP8_SCALE * FP8_SCALE)

    nc = bass.Bass(num_devices=NCORES)
    h1 = nc.dram_tensor("h1", [ncomp, D], F32, kind="ExternalInput")
    h2 = nc.dram_tensor("h2", [ncomp, D], F32, kind="ExternalInput")
    # aux: cols 0..st-1 = mask in [128, st] layout (token t = 128*col+row),
    # col st = negK0 = 2n - 2*ncomp (broadcast down the partition dim)
    aux = nc.dram_tensor("aux", [128, st + 1], F32, kind="ExternalInput")
    out = nc.dram_tensor("loss", [1, 1], F32, kind="ExternalOutput")

    with tile.TileContext(nc) as tc, ExitStack() as ctx:
        const_pool = ctx.enter_context(tc.tile_pool(name="const", bufs=1))
        big = ctx.enter_context(tc.tile_pool(name="big", bufs=1))
        stat = ctx.enter_context(tc.tile_pool(name="stat", bufs=1))

        ident = const_pool.tile([128, 128], BF16)
        make_identity(nc, ident[:])
        negbigI = const_pool.tile([128, 128], BF16)
        nc.vector.tensor_scalar_mul(negbigI[:], ident[:], NEGBIG)
        ones_bf = const_pool.tile([128, 1], BF16)
        nc.gpsimd.memset(ones_bf[:], 1.0)
        ones_f = const_pool.tile([128, 1], F32)
        nc.gpsimd.memset(ones_f[:], 1.0)
        ln8_col = const_pool.tile([128, 1], F32)
        nc.gpsimd.memset(ln8_col[:], float(np.log(FP8_SCALE)))
        auxs = const_pool.tile([128, st + 1], F32)
        nc.sync.dma_start(auxs[:], aux[:])
        msk = auxs[:, 0:st]
        negK0 = auxs[:, st:st + 1]

        h1b = big.tile([128, st, D], F32)
        h2b = big.tile([128, st, D], F32)
        fT1 = big.tile([128, KT, ncomp], FP8)     # f1^T * 8
        fT2 = big.tile([128, KT, ncomp], FP8)
        ss = stat.tile([128, nb], F32)            # |h|^2 per token, both views
        sc12 = stat.tile([128, nb], F32)          # 8 * mask * rsqrt(ss)
        accA = stat.tile([128, st], F32)          # A-quadrant row sums
        accB = stat.tile([128, nb], F32)          # B+D quadrant row sums
        pose = stat.tile([128, st], F32)          # exp(pos_sim/T) extracted
        cacc = stat.tile([128, st], F32)          # B col sums (C row sums)

        # ---- load + norms (squares), both views ----
        with tc.tile_pool(name="sq", bufs=2) as sqp:
            for v, (hd, hb) in enumerate(((h1, h1b), (h2, h2b))):
                for t in range(st):
                    nc.sync.dma_start(hb[:, t, :], hd[t * 128:(t + 1) * 128, :])
                    sq = sqp.tile([128, D], BF16, tag="sq", name=f"sq{v}_{t}")
                    nc.scalar.activation(sq[:], hb[:, t, :], AF.Square,
                                         accum_out=ss[:, v * st + t:v * st + t + 1])
        # sc = 8 * mask * rsqrt(ss) = mask * exp(-0.5*ln(ss) + ln(8))
        lnss = stat.tile([128, nb], F32)
        nc.scalar.activation(lnss[:], ss[:], AF.Ln)
        rinv8 = stat.tile([128, nb], F32)
        nc.scalar.activation(rinv8[:], lnss[:], AF.Exp, scale=-0.5,
                             bias=ln8_col[:])
        nc.vector.tensor_mul(sc12[:, 0:st], rinv8[:, 0:st], msk)
        nc.vector.tensor_mul(sc12[:, st:nb], rinv8[:, st:nb], msk)

        # ---- normalize + transpose -> fT (fp8, [D, ncomp] per view) ----
        with tc.tile_pool(name="fn", bufs=2) as fnp, \
             tc.tile_pool(name="tp_ps", bufs=2, space="PSUM") as tps:
            for v, (hb, fT) in enumerate(((h1b, fT1), (h2b, fT2))):
                for t in range(st):
                    fn = fnp.tile([128, D], BF16, tag="fn", name=f"fn{v}_{t}")
                    nc.vector.tensor_scalar_mul(
                        fn[:], hb[:, t, :], sc12[:, v * st + t:v * st + t + 1])
                    pt = tps.tile([128, D], BF16, tag="pt", name=f"pt{v}_{t}")
                    for k in range(KT):
                        nc.tensor.transpose(pt[:, k * 128:(k + 1) * 128],
                                            fn[:, k * 128:(k + 1) * 128],
                                            ident[:])
                    nc.vector.tensor_copy(
                        fT[:, :, t * 128:(t + 1) * 128],
                        pt[:].rearrange("p (k c) -> p k c", k=KT))

        # ---- sim strips: 3 half-rows of row passes ----
        def row_pass(mmp, esp, r, lhsT, rhsT, dk_block, acc_col, want_cacc,
                     want_pos, cacc_ps, first_cacc, last_cacc):
            rT = r % st
            ps = mmp.tile([128, 1024 * ((ncomp + 1023) // 1024)], F32,
                          tag="ps", name=f"ps_{r}_{acc_col[1]}")
            for g in range(KT // 2):
                for (c0, w) in strips:
                    nc.tensor.matmul(
                        ps[:, c0:c0 + w],
                        lhsT[:, 2 * g:2 * g + 2, rT * 128:(rT + 1) * 128],
                        rhsT[:, 2 * g:2 * g + 2, c0:c0 + w],
                        perf_mode=mybir.MatmulPerfMode.DoubleRow,
                        start=(g == 0),
                        stop=(g == KT // 2 - 1 and dk_block is None))
            if dk_block is not None:
                b0 = dk_block * 128
                nc.tensor.matmul(ps[:, b0:b0 + 128], negbigI[:], ident[:],
                                 start=False, stop=True, skip_group_check=True)
            es = esp.tile([128, ncomp], BF16, tag="es", name=f"es_{r}_{acc_col[1]}")
            nc.scalar.activation(es[:], ps[:, 0:ncomp], AF.Exp,
                                 scale=exp_scale,
                                 accum_out=acc_col[0][:, acc_col[1]:acc_col[1] + 1])
            if want_cacc:
                for ci, (c0, w) in enumerate(strips):
                    nc.tensor.matmul(
                        cacc_ps[32 * ci:32 * ci + 1, 0:w],
                        ones_bf[:], es[:, c0:c0 + w],
                        start=first_cacc, stop=last_cacc,
                        skip_group_check=True)
            if want_pos:
                psel = esp.tile([128, 128], BF16, tag="psel", name=f"psel_{r}")
                blk = es[:, rT * 128:(rT + 1) * 128]
                nc.gpsimd.affine_select(
                    out=psel[:], in_=blk, compare_op=ALU.is_equal,
                    fill=0.0, base=0, pattern=[[-1, 128]],
                    channel_multiplier=1)
                nc.vector.tensor_reduce(pose[:, rT:rT + 1], psel[:],
                                        axis=mybir.AxisListType.X, op=ALU.add)

        mm_bufs = 2 if ncomp <= 1024 else 1
        with tc.tile_pool(name="mm_ps", bufs=mm_bufs, space="PSUM") as mmp, \
             tc.tile_pool(name="es", bufs=3) as esp, \
             tc.tile_pool(name="cacc_ps", bufs=1, space="PSUM") as cbp:
            cacc_ps = cbp.tile([128, 512], F32)
            # A quadrant: view-1 rows x view-1 cols; kill self-diag
            for r in range(st):
                row_pass(mmp, esp, r, fT1, fT1, r, (accA, r),
                         False, False, None, False, False)
            # B quadrant: view-1 rows x view-2 cols; pos diag stays in the
            # sum; accumulate column sums; extract pos
            for r in range(st):
                row_pass(mmp, esp, r, fT1, fT2, None, (accB, r),
                         True, True, cacc_ps, r == 0, r == st - 1)
            # D quadrant: view-2 rows x view-2 cols; kill self-diag
            for r in range(st, nb):
                row_pass(mmp, esp, r, fT2, fT2, r % st, (accB, r),
                         False, False, None, False, False)

            # cacc [1, ncomp] -> token layout [128, st] via PE transposes
            crow = esp.tile([1, ncomp], F32, tag="crow", name="crow")
            for ci, (c0, w) in enumerate(strips):
                nc.vector.tensor_copy(crow[:, c0:c0 + w],
                                      cacc_ps[32 * ci:32 * ci + 1, 0:w])
            with tc.tile_pool(name="ct_ps", bufs=1, space="PSUM") as ctp:
                ct = ctp.tile([128, st], F32)
                for c in range(st):
                    # rank-1 matmul: ct[:, c] = crow[0, 128c:128c+128]^T * 1
                    nc.tensor.matmul(ct[:, c:c + 1],
                                     crow[0:1, c * 128:(c + 1) * 128],
                                     ones_f[0:1, :], start=True, stop=True)
                nc.vector.tensor_copy(cacc[:], ct[:])

        # ---- epilogue: per-core masked sum of ln(Ng+pos) - ln(pos) ----
        with tc.tile_pool(name="ep", bufs=1) as ep, \
             tc.tile_pool(name="ep_ps", bufs=1, space="PSUM") as epp:
            ng = ep.tile([128, nb], F32)
            nc.vector.tensor_add(ng[:, 0:st], accA[:], accB[:, 0:st])
            nc.vector.tensor_add(ng[:, st:nb], accB[:, st:nb], cacc[:])
            lg = ep.tile([128, nb], F32)
            # ln(rowsum + negK0) = ln(Ng + pos)
            nc.scalar.activation(lg[:], ng[:], AF.Ln, bias=negK0)
            plog = ep.tile([128, st], F32)
            nc.scalar.activation(plog[:], pose[:], AF.Ln)
            ptok = ep.tile([128, nb], F32)
            nc.vector.tensor_sub(ptok[:, 0:st], lg[:, 0:st], plog[:])
            nc.vector.tensor_sub(ptok[:, st:nb], lg[:, st:nb], plog[:])
            nc.vector.tensor_mul(ptok[:, 0:st], ptok[:, 0:st], msk)
            nc.vector.tensor_mul(ptok[:, st:nb], ptok[:, st:nb], msk)
            tsum = ep.tile([128, 1], F32)
            nc.vector.tensor_reduce(tsum[:], ptok[:],
                                    axis=mybir.AxisListType.X, op=ALU.add)
            lps = epp.tile([1, 1], F32)
            nc.tensor.matmul(lps[:], ones_f[:], tsum[:], start=True, stop=True)
            lsb = ep.tile([1, 1], F32)
            nc.vector.tensor_copy(lsb[:], lps[:])
            nc.sync.dma_start(out[:], lsb[:])

    return nc


_NC_CACHE: dict = {}


def _get_nc(ncomp: int) -> bass.Bass:
    if ncomp not in _NC_CACHE:
        _NC_CACHE[ncomp] = _build(ncomp)
    return _NC_CACHE[ncomp]


def _prep_core(h1_b: np.ndarray, h2_b: np.ndarray, mask_b: np.ndarray,
               ncomp: int):
    idx = np.nonzero(mask_b)[0]
    n = idx.shape[0]
    idx_pad = np.zeros(ncomp, dtype=np.int64)
    idx_pad[:n] = idx
    st = ncomp // 128
    maskc = np.zeros(ncomp, dtype=np.float32)
    maskc[:n] = 1.0
    aux = np.empty((128, st + 1), dtype=np.float32)
    aux[:, 0:st] = maskc.reshape(st, 128).T
    aux[:, st] = 2.0 * n - 2.0 * ncomp
    return ({"h1": np.ascontiguousarray(h1_b[idx_pad], dtype=np.float32),
             "h2": np.ascontiguousarray(h2_b[idx_pad], dtype=np.float32),
             "aux": aux}, n)


def _in_maps(last_hidden_states_1, last_hidden_states_2, token_mask_batch):
    h1 = np.asarray(last_hidden_states_1, dtype=np.float32)
    h2 = np.asarray(last_hidden_states_2, dtype=np.float32)
    mask = np.asarray(token_mask_batch).astype(bool)
    assert h1.shape == (NCORES, S, D), h1.shape
    max_n = int(mask.sum(axis=1).max())
    ncomp = max(NCOMP, -(-max_n // 128) * 128)
    maps, ns = [], []
    for b in range(NCORES):
        m, n = _prep_core(h1[b], h2[b], mask[b], ncomp)
        maps.append(m)
        ns.append(n)
    return maps, ns, ncomp


def kernel(last_hidden_states_1, last_hidden_states_2, token_mask_batch):
    maps, ns, ncomp = _in_maps(last_hidden_states_1, last_hidden_states_2,
                               token_mask_batch)
    nc = _get_nc(ncomp)
    res = run_bass_kernel_spmd(nc, maps, list(range(NCORES)))
    per_core = [
        float(np.asarray(res.results[b]["loss"], dtype=np.float32).reshape(()))
        / (2.0 * ns[b])
        for b in range(NCORES)
    ]
    return np.float32(np.mean(per_core))


# revision 8
# speedup vs baseline: 2.5821x; 1.1718x over previous
"""ContraCLM token-level contrastive loss on 8 Trainium2 NeuronCores.

Data-parallel over the batch: core b handles sample b (B=8). The token
mask is known on the host inside kernel(), so unmasked tokens are
COMPACTED host-side to a fixed padded length NCOMP (= 896 >= n + 6.5
sigma for n ~ Binom(1536, 1/2)); pad slots reuse token 0's row and get
mask 0. The device then works on a dense [2*NCOMP, 2*NCOMP] problem --
2.9x fewer matmul/exp elements than the full 2S grid.

Per core, with N_c = NCOMP, D = 1024, T = 0.05:

  f_v = l2norm(h_v) masked; rsqrt computed as exp(-0.5*ln(ss)) so the
  ScalarE only ever needs the natural_log_exp activation-table set.
  F = [f1; f2], stored transposed as fp8 [D, 2*N_c] (x8 scale).

  sim strips: per row-block r, one PSUM group [128, N_c] (<=2 banks)
  accumulates 4 DoubleRow fp8 matmuls per 512-col strip (K=1024).
  Self-similarity diagonals are killed by ONE extra tiny matmul that
  accumulates -30000*I into the diagonal 128-block, so exp() of the
  whole group rowsum-accumulates on the ScalarE free-dim accumulator
  with no fixups. The positive-pair column (block diagonal of the B
  quadrant) stays IN the row sum (denominator = Ng + pos), and pos
  itself is extracted from the exp'd block via affine_select(diag).

  C-quadrant row sums (view-2 rows vs view-1 cols) are B-quadrant
  column sums by symmetry: ones-weight matmuls accumulate [1, 512]
  PSUM rows which are transposed back to token layout on the PE.

  Masked/pad columns contribute exp(0)=1 to every row sum: corrected
  with negK0 = 2n - 2*N_c (host-computed, folded into the Ln bias).
  per-core output = sum_tok mask*(ln(Ng+pos) - ln(pos)); the host
  divides by 2n and averages across the 8 cores (no device collective).
"""

import sys

for _p in ("/opt/trn_rl_repo", "/opt/pypackages"):
    if _p not in sys.path:
        sys.path.append(_p)

from contextlib import ExitStack

import numpy as np

import bass_rust

import concourse.bass as bass
import concourse.tile as tile
from concourse import mybir
from concourse.bass_utils import run_bass_kernel_spmd
from concourse.masks import make_identity
from concourse.vector_clock import ScopedClock

# The walrus build in this container encodes at most 2 sync waits per
# instruction (bass_rust's inst_waits_full agrees), but Tile's semaphore
# assignment can attach more. Hoist excess waits onto unfusable same-engine
# NoOps immediately before the instruction -- the engine executes its queue
# in order, so semantics are preserved.
_MAX_WAITS = 1


def _split_excess_waits(nc, ordered):
    for bb_name, insts in ordered.items():
        out = []
        changed = False
        for inst in insts:
            si = getattr(inst, "sync_info", None)
            waits = list(si.on_wait) if si is not None else []
            if len(waits) > _MAX_WAITS:
                changed = True
                extra, keep = waits[:-_MAX_WAITS], waits[-_MAX_WAITS:]
                for i in range(0, len(extra), _MAX_WAITS):
                    out.append(mybir.InstNoOp(
                        name=nc.get_next_instruction_name(),
                        sync_info=mybir.SyncInfo(
                            on_wait=extra[i:i + _MAX_WAITS], on_update=[]),
                        bass_nofuse=True,
                        engine=inst.engine,
                    ))
                si.on_wait = keep
            out.append(inst)
        if changed:
            insts[:] = out


_orig_lower_ordered_insts = tile.TileContext._lower_ordered_insts


def _patched_lower_ordered_insts(self, ordered):
    _split_excess_waits(self.nc, ordered)
    return _orig_lower_ordered_insts(self, ordered)


tile.TileContext._lower_ordered_insts = _patched_lower_ordered_insts


def _split_waits_drain_and_barrier(self, tick_clock, wait_clock):
    nc = self.nc
    probe = nc.sync.nop(nofuse=True)
    wait_clock.add_sem_waits(
        probe.ins, ScopedClock({None: tick_clock.global_clock}))
    si = probe.ins.sync_info
    waits = list(si.on_wait) if si is not None else []
    if len(waits) > _MAX_WAITS:
        si.on_wait = waits[:_MAX_WAITS]
        for i in range(_MAX_WAITS, len(waits), _MAX_WAITS):
            nxt = nc.sync.nop(nofuse=True)
            nxt.ins.sync_info = bass_rust.SyncInfo(
                on_wait=waits[i:i + _MAX_WAITS], on_update=[])
    nc.sync.drain()
    nc.all_engine_barrier()
    assert self.sems is not None
    popped = nc._tile_sem_poison_stack.pop()
    assert popped is self._sem_poison
    nc.clear_and_free_semaphores(list(self.sems.allocated().values()))
    nc.all_engine_barrier()


tile.TileContext._drain_and_barrier = _split_waits_drain_and_barrier

S, D, NCORES = 1536, 1024, 8
NCOMP = 896              # padded compacted token count (multiple of 128)
KT = D // 128            # 8 contraction k-tiles
TEMP_INV = 20.0          # 1 / 0.05
FP8_SCALE = 8.0          # f entries ~N(0, 1/32); x8 keeps them in e4m3 range
NEGBIG = -30000.0        # diag killer: exp(scale*(64*sim + NEGBIG)) == 0
F32 = mybir.dt.float32
BF16 = mybir.dt.bfloat16
FP8 = mybir.dt.float8e4
AF = mybir.ActivationFunctionType
ALU = mybir.AluOpType


def _col_strips(sc):
    """Split sc (= NCOMP, multiple of 128) into <=512-wide col strips."""
    strips = []
    c = 0
    while c < sc:
        w = min(512, sc - c)
        strips.append((c, w))
        c += w
    return strips


def _build(ncomp: int) -> bass.Bass:
    st = ncomp // 128          # token tiles per view
    nb = 2 * st                # row blocks of F
    strips = _col_strips(ncomp)
    exp_scale = TEMP_INV / (FP8_SCALE * FP8_SCALE)

    nc = bass.Bass(num_devices=NCORES)
    h1 = nc.dram_tensor("h1", [ncomp, D], F32, kind="ExternalInput")
    h2 = nc.dram_tensor("h2", [ncomp, D], F32, kind="ExternalInput")
    # aux: cols 0..st-1 = mask in [128, st] layout (token t = 128*col+row),
    # col st = negK0 = 2n - 2*ncomp (broadcast down the partition dim)
    aux = nc.dram_tensor("aux", [128, st + 1], F32, kind="ExternalInput")
    out = nc.dram_tensor("loss", [1, 1], F32, kind="ExternalOutput")

    with tile.TileContext(nc) as tc, ExitStack() as ctx:
        const_pool = ctx.enter_context(tc.tile_pool(name="const", bufs=1))
        big = ctx.enter_context(tc.tile_pool(name="big", bufs=1))
        stat = ctx.enter_context(tc.tile_pool(name="stat", bufs=1))

        ident = const_pool.tile([128, 128], BF16)
        make_identity(nc, ident[:])
        negbigI = const_pool.tile([128, 128], BF16)
        nc.vector.tensor_scalar_mul(negbigI[:], ident[:], NEGBIG)
        ones_bf = const_pool.tile([128, 1], BF16)
        nc.gpsimd.memset(ones_bf[:], 1.0)
        ones_f = const_pool.tile([128, 1], F32)
        nc.gpsimd.memset(ones_f[:], 1.0)
        ln8_col = const_pool.tile([128, 1], F32)
        nc.gpsimd.memset(ln8_col[:], float(np.log(FP8_SCALE)))
        auxs = const_pool.tile([128, st + 1], F32)
        nc.sync.dma_start(auxs[:], aux[:])
        msk = auxs[:, 0:st]
        negK0 = auxs[:, st:st + 1]

        h1b = big.tile([128, st, D], F32)
        h2b = big.tile([128, st, D], F32)
        fT1 = big.tile([128, KT, ncomp], FP8)     # f1^T * 8
        fT2 = big.tile([128, KT, ncomp], FP8)
        ss = stat.tile([128, nb], F32)            # |h|^2 per token, both views
        sc12 = stat.tile([128, nb], F32)          # 8 * mask * rsqrt(ss)
        accA = stat.tile([128, st], F32)          # A-quadrant row sums
        accB = stat.tile([128, nb], F32)          # B+D quadrant row sums
        pose = stat.tile([128, st], F32)          # exp(pos_sim/T) extracted
        cacc = stat.tile([128, st], F32)          # B col sums (C row sums)

        # ---- sim strips: 3 half-rows of row passes ----
        def row_pass(mmp, esp, r, lhsT, rhsT, dk_block, acc_col, want_cacc,
                     want_pos, cacc_ps, first_cacc, last_cacc):
            rT = r % st
            ps = mmp.tile([128, 1024 * ((ncomp + 1023) // 1024)], F32,
                          tag="ps", name=f"ps_{r}_{acc_col[1]}")
            for g in range(KT // 2):
                for (c0, w) in strips:
                    nc.tensor.matmul(
                        ps[:, c0:c0 + w],
                        lhsT[:, 2 * g:2 * g + 2, rT * 128:(rT + 1) * 128],
                        rhsT[:, 2 * g:2 * g + 2, c0:c0 + w],
                        perf_mode=mybir.MatmulPerfMode.DoubleRow,
                        start=(g == 0),
                        stop=(g == KT // 2 - 1 and dk_block is None))
            if dk_block is not None:
                b0 = dk_block * 128
                nc.tensor.matmul(ps[:, b0:b0 + 128], negbigI[:], ident[:],
                                 start=False, stop=True, skip_group_check=True)
            es = esp.tile([128, ncomp], BF16, tag="es", name=f"es_{r}_{acc_col[1]}")
            nc.scalar.activation(es[:], ps[:, 0:ncomp], AF.Exp,
                                 scale=exp_scale,
                                 accum_out=acc_col[0][:, acc_col[1]:acc_col[1] + 1])
            if want_cacc:
                for ci, (c0, w) in enumerate(strips):
                    nc.tensor.matmul(
                        cacc_ps[32 * ci:32 * ci + 1, 0:w],
                        ones_bf[:], es[:, c0:c0 + w],
                        start=first_cacc, stop=last_cacc,
                        skip_group_check=True)
            if want_pos:
                psel = esp.tile([128, 128], BF16, tag="psel", name=f"psel_{r}")
                blk = es[:, rT * 128:(rT + 1) * 128]
                nc.gpsimd.affine_select(
                    out=psel[:], in_=blk, compare_op=ALU.is_equal,
                    fill=0.0, base=0, pattern=[[-1, 128]],
                    channel_multiplier=1)
                nc.vector.tensor_reduce(pose[:, rT:rT + 1], psel[:],
                                        axis=mybir.AxisListType.X, op=ALU.add)

        mm_bufs = 2 if ncomp <= 1024 else 1
        with tc.tile_pool(name="mm_ps", bufs=mm_bufs, space="PSUM") as mmp, \
             tc.tile_pool(name="es", bufs=3) as esp, \
             tc.tile_pool(name="cacc_ps", bufs=1, space="PSUM") as cbp, \
             tc.tile_pool(name="fn", bufs=2) as fnp, \
             tc.tile_pool(name="nrm", bufs=4) as nrm, \
             tc.tile_pool(name="tp_ps", bufs=2, space="PSUM") as tps:
            cacc_ps = cbp.tile([128, 512], F32)

            def load_view(v, hd, hb, fT):
                """Per-tile pipelined DMA -> norm -> scale -> transpose."""
                for t in range(st):
                    col = v * st + t
                    nc.sync.dma_start(hb[:, t, :], hd[t * 128:(t + 1) * 128, :])
                    sq = fnp.tile([128, D], BF16, tag="sq", name=f"sq{v}_{t}")
                    nc.scalar.activation(sq[:], hb[:, t, :], AF.Square,
                                         accum_out=ss[:, col:col + 1])
                    lncol = nrm.tile([128, 1], F32, tag="ln", name=f"ln{v}_{t}")
                    nc.scalar.activation(lncol[:], ss[:, col:col + 1], AF.Ln)
                    # 8 * rsqrt(ss) = exp(-0.5*ln(ss) + ln(8))
                    rcol = nrm.tile([128, 1], F32, tag="ri", name=f"ri{v}_{t}")
                    nc.scalar.activation(rcol[:], lncol[:], AF.Exp,
                                         scale=-0.5, bias=ln8_col[:])
                    nc.vector.tensor_mul(sc12[:, col:col + 1], rcol[:],
                                         msk[:, t:t + 1])
                    fn = fnp.tile([128, D], BF16, tag="fn", name=f"fn{v}_{t}")
                    nc.vector.tensor_scalar_mul(
                        fn[:], hb[:, t, :], sc12[:, col:col + 1])
                    pt = tps.tile([128, D], BF16, tag="pt", name=f"pt{v}_{t}")
                    for k in range(KT):
                        nc.tensor.transpose(pt[:, k * 128:(k + 1) * 128],
                                            fn[:, k * 128:(k + 1) * 128],
                                            ident[:])
                    nc.vector.tensor_copy(
                        fT[:, :, t * 128:(t + 1) * 128],
                        pt[:].rearrange("p (k c) -> p k c", k=KT))

            load_view(0, h1, h1b, fT1)
            # A quadrant: view-1 rows x view-1 cols; kill self-diag.
            # Emitted before view-2 prep so the PE overlaps it with the
            # view-2 DMA/norm chain.
            for r in range(st):
                row_pass(mmp, esp, r, fT1, fT1, r, (accA, r),
                         False, False, None, False, False)
            load_view(1, h2, h2b, fT2)
            # B quadrant: view-1 rows x view-2 cols; pos diag stays in the
            # sum; accumulate column sums; extract pos
            for r in range(st):
                row_pass(mmp, esp, r, fT1, fT2, None, (accB, r),
                         True, True, cacc_ps, r == 0, r == st - 1)

            # cacc [1, ncomp] -> token layout [128, st]; overlaps D rows
            crow = esp.tile([1, ncomp], F32, tag="crow", name="crow")
            for ci, (c0, w) in enumerate(strips):
                nc.vector.tensor_copy(crow[:, c0:c0 + w],
                                      cacc_ps[32 * ci:32 * ci + 1, 0:w])
            with tc.tile_pool(name="ct_ps", bufs=1, space="PSUM") as ctp:
                ct = ctp.tile([128, st], F32)
                for c in range(st):
                    # rank-1 matmul: ct[:, c] = crow[0, 128c:128c+128]^T * 1
                    nc.tensor.matmul(ct[:, c:c + 1],
                                     crow[0:1, c * 128:(c + 1) * 128],
                                     ones_f[0:1, :], start=True, stop=True)
                nc.vector.tensor_copy(cacc[:], ct[:])

                # D quadrant: view-2 rows x view-2 cols; kill self-diag
                for r in range(st, nb):
                    row_pass(mmp, esp, r, fT2, fT2, r % st, (accB, r),
                             False, False, None, False, False)

        # ---- epilogue: per-core masked sum of ln(Ng+pos) - ln(pos) ----
        with tc.tile_pool(name="ep", bufs=1) as ep, \
             tc.tile_pool(name="ep_ps", bufs=1, space="PSUM") as epp:
            ng = ep.tile([128, nb], F32)
            nc.vector.tensor_add(ng[:, 0:st], accA[:], accB[:, 0:st])
            nc.vector.tensor_add(ng[:, st:nb], accB[:, st:nb], cacc[:])
            lg = ep.tile([128, nb], F32)
            # ln(rowsum + negK0) = ln(Ng + pos)
            nc.scalar.activation(lg[:], ng[:], AF.Ln, bias=negK0)
            plog = ep.tile([128, st], F32)
            nc.scalar.activation(plog[:], pose[:], AF.Ln)
            ptok = ep.tile([128, nb], F32)
            nc.vector.tensor_sub(ptok[:, 0:st], lg[:, 0:st], plog[:])
            nc.vector.tensor_sub(ptok[:, st:nb], lg[:, st:nb], plog[:])
            nc.vector.tensor_mul(ptok[:, 0:st], ptok[:, 0:st], msk)
            nc.vector.tensor_mul(ptok[:, st:nb], ptok[:, st:nb], msk)
            tsum = ep.tile([128, 1], F32)
            nc.vector.tensor_reduce(tsum[:], ptok[:],
                                    axis=mybir.AxisListType.X, op=ALU.add)
            lps = epp.tile([1, 1], F32)
            nc.tensor.matmul(lps[:], ones_f[:], tsum[:], start=True, stop=True)
            lsb = ep.tile([1, 1], F32)
            nc.vector.tensor_copy(lsb[:], lps[:])
            nc.sync.dma_start(out[:], lsb[:])

    return nc


_NC_CACHE: dict = {}


def _get_nc(ncomp: int) -> bass.Bass:
    if ncomp not in _NC_CACHE:
        _NC_CACHE[ncomp] = _build(ncomp)
    return _NC_CACHE[ncomp]


def _prep_core(h1_b: np.ndarray, h2_b: np.ndarray, mask_b: np.ndarray,
               ncomp: int):
    idx = np.nonzero(mask_b)[0]
    n = idx.shape[0]
    idx_pad = np.zeros(ncomp, dtype=np.int64)
    idx_pad[:n] = idx
    st = ncomp // 128
    maskc = np.zeros(ncomp, dtype=np.float32)
    maskc[:n] = 1.0
    aux = np.empty((128, st + 1), dtype=np.float32)
    aux[:, 0:st] = maskc.reshape(st, 128).T
    aux[:, st] = 2.0 * n - 2.0 * ncomp
    return ({"h1": np.ascontiguousarray(h1_b[idx_pad], dtype=np.float32),
             "h2": np.ascontiguousarray(h2_b[idx_pad], dtype=np.float32),
             "aux": aux}, n)


def _in_maps(last_hidden_states_1, last_hidden_states_2, token_mask_batch):
    h1 = np.asarray(last_hidden_states_1, dtype=np.float32)
    h2 = np.asarray(last_hidden_states_2, dtype=np.float32)
    mask = np.asarray(token_mask_batch).astype(bool)
    assert h1.shape == (NCORES, S, D), h1.shape
    max_n = int(mask.sum(axis=1).max())
    ncomp = max(NCOMP, -(-max_n // 128) * 128)
    maps, ns = [], []
    for b in range(NCORES):
        m, n = _prep_core(h1[b], h2[b], mask[b], ncomp)
        maps.append(m)
        ns.append(n)
    return maps, ns, ncomp


def kernel(last_hidden_states_1, last_hidden_states_2, token_mask_batch):
    maps, ns, ncomp = _in_maps(last_hidden_states_1, last_hidden_states_2,
                               token_mask_batch)
    nc = _get_nc(ncomp)
    res = run_bass_kernel_spmd(nc, maps, list(range(NCORES)))
    per_core = [
        float(np.asarray(res.results[b]["loss"], dtype=np.float32).reshape(()))
        / (2.0 * ns[b])
        for b in range(NCORES)
    ]
    return np.float32(np.mean(per_core))
